# revision 27
# baseline (speedup 1.0000x reference)
"""HypergraphConv (PyG, use_attention=False) Trainium2 kernel, 8 NeuronCores.

  out = D^-1 H B^-1 H^T X W + b

Edges are partitioned across the 8 cores for the node->edge aggregation
(stage 1), nodes for the edge->node aggregation (stage 2); the per-core edge
features are exchanged between the stages with two bf16 AllGathers (one per
half of the edge shard, so the first can overlap the rest of stage 1).

Both segment-sums run on the tensor engine as one-hot matmuls over rows
fetched with indirect DMA (dma_gather).  Numerics are bf16 single-precision
(no hi/lo split): inputs round to bf16 once and every aggregation accumulates
in fp32 PSUM, which lands ~4e-3 max-norm error, well under the 2e-2 gate.
The B^-1 / D^-1 row scalings ride the scalar-engine PSUM->SBUF copies
(activation with a per-partition scale operand), W is applied on the edge
shard (6250 rows/core instead of 12500), and the bias uses a rank-1
deg-scaled matmul so it survives the later D^-1 scaling.

dma_gather descriptor generation on the GPSIMD engine is the critical
resource (~8.5 ns/row on one SWDGE queue); calls are round-robined over all
4 SWDGE queues, which runs generation concurrently (~2.4 us per 1024-row
call), and each gathered row is fetched exactly once (bf16-only halves the
descriptor count vs hi/lo).  Stage 2 prefetches its first range-0 gathers
before any range-1 gather so the pool engine stays busy while the
second-half AllGather is still in flight.
"""

import sys
from contextlib import ExitStack

import numpy as np

for _p in ("/opt/trn_rl_repo", "/root/.axon_site/_ro/trn_rl_repo"):
    if _p not in sys.path:
        sys.path.insert(0, _p)

import ml_dtypes  # noqa: E402

BF16 = ml_dtypes.bfloat16


class Cfg:
    def __init__(self, NN=100000, NE=50000, NNZ=500000, F=128, C=8,
                 R1_SZ=25000):
        self.NN, self.NE, self.NNZ, self.F, self.C = NN, NE, NNZ, F, C
        self.EPC = NE // C                      # edges per core
        self.NPC = NN // C                      # nodes per core
        self.EW = (self.EPC + 127) // 128       # edge windows per core
        self.NW = (self.NPC + 127) // 128       # node windows per core
        self.R1_SZ = R1_SZ                      # node range size (int16 limit)
        self.R1 = (NN + R1_SZ - 1) // R1_SZ
        # stage-1 output halves (separate AllGathers)
        self.EWH0 = (self.EW + 1) // 2          # windows in half 0
        self.EWH1 = self.EW - self.EWH0
        self.EFP0 = self.EWH0 * 128             # padded rows, half 0
        self.EFP1 = self.EWH1 * 128
        self.R2 = 2                             # stage-2 tables = the 2 halves
        assert self.R1_SZ <= 32767
        assert C * self.EFP0 <= 32767 and C * self.EFP1 <= 32767
        self.WB1 = 4                            # windows per batch
        self.WB2 = 4
        self.PRE2 = 4                           # stage-2 prefetch batches


FULL = Cfg()


def _wrap_idx(vals):
    """int16 index layout for dma_gather: [128, n/16], A[16k+p, j]=idx[16j+p]."""
    n = vals.shape[-1]
    assert n % 16 == 0
    a = vals.reshape(-1, n // 16, 16)
    a = np.swapaxes(a, -1, -2)
    return np.tile(a, (1, 8, 1)).astype(np.int16)


def _group_streams(cfg, seg_c, seg_w, seg_r, idxval, segval, sclval,
                   NRANGE, NWIN, order):
    """Build padded per-(core, range) slot streams.

    Entries (already lexsorted by (c, w, r, segment)) are laid out per core
    into NRANGE streams; within a stream, each window's entries are padded to
    a multiple of 128 slots.  Chunk counts per (window, range) are the max
    over cores so the SPMD program is identical on every core.  Padding slots
    gather row 0 with locseg=-1 and scale 0 -> all-zero one-hot column.
    """
    C = cfg.C
    key = (seg_c * NWIN + seg_w) * NRANGE + seg_r
    cnt = np.bincount(key, minlength=C * NWIN * NRANGE).reshape(C, NWIN, NRANGE)
    M = np.maximum(1, -(-cnt.max(axis=0) // 128))        # [NWIN, NRANGE]
    slots = M * 128
    base = np.zeros((NWIN, NRANGE), np.int64)
    base[1:] = np.cumsum(slots, axis=0)[:-1]
    L = slots.sum(axis=0)                                # [NRANGE]

    so = order
    sk = key[so]
    grp_change = np.r_[True, sk[1:] != sk[:-1]]
    grp_start = np.flatnonzero(grp_change)
    grp_len = np.diff(np.r_[grp_start, len(sk)])
    rank = np.arange(len(sk)) - np.repeat(grp_start, grp_len)

    pos = base[seg_w[so], seg_r[so]] + rank
    idx_s = [np.zeros((C, int(L[r])), np.int64) for r in range(NRANGE)]
    seg_s = [np.full((C, int(L[r])), -1.0, np.float32) for r in range(NRANGE)]
    scl_s = [np.zeros((C, int(L[r])), np.float32) for r in range(NRANGE)]
    c_s, r_s = seg_c[so], seg_r[so]
    iv, sv, cv = idxval[so], segval[so], sclval[so]
    for r in range(NRANGE):
        m = r_s == r
        idx_s[r][c_s[m], pos[m]] = iv[m]
        seg_s[r][c_s[m], pos[m]] = sv[m]
        scl_s[r][c_s[m], pos[m]] = cv[m]
    return M, base, idx_s, seg_s, scl_s


def host_prep(cfg, x, hyperedge_index, W, b):
    C, F = cfg.C, cfg.F
    ni = hyperedge_index[0].astype(np.int64)
    ei = hyperedge_index[1].astype(np.int64)
    x = np.asarray(x, np.float32)

    deg_n = np.bincount(ni, minlength=cfg.NN).astype(np.float32)
    deg_e = np.bincount(ei, minlength=cfg.NE).astype(np.float32)
    with np.errstate(divide="ignore"):
        d_inv = np.where(deg_n > 0, 1.0 / deg_n, 0.0).astype(np.float32)
        b_inv = np.where(deg_e > 0, 1.0 / deg_e, 0.0).astype(np.float32)

    x_hi = x.astype(BF16)

    # ---- stage 1: aggregate x rows by edge (edge partition) ----
    c1 = ei // cfg.EPC
    w1 = (ei % cfg.EPC) // 128

    # pick ceil-aware range boundaries: per-(window, range) entry counts pad
    # to multiples of 128, so skewing the node-range sizes (each still within
    # the int16 gather-index limit) can drop a whole chunk per window
    def split_cost(starts):
        ends = np.r_[starts[1:], cfg.NN]
        rr = np.searchsorted(ends, ni, side="right")
        key = (c1 * cfg.EW + w1) * cfg.R1 + rr
        cnt = np.bincount(key, minlength=cfg.C * cfg.EW * cfg.R1)
        cnt = cnt.reshape(cfg.C, cfg.EW, cfg.R1)
        return int(np.maximum(1, -(-cnt.max(axis=0) // 128)).sum())

    lim = 32767
    cands = []
    for f3 in (0.25, 0.22, 0.19, 0.16, 0.13, 0.10):
        szs = [min(lim, int(cfg.NN * (1 - f3) / 3 + 0.5))] * 3
        szs = szs[:3] + [cfg.NN - sum(szs)]
        if all(0 < s <= lim for s in szs):
            cands.append(np.r_[0, np.cumsum(szs)[:-1]].astype(np.int64))
    best = min(cands, key=split_cost)
    r1_starts = best
    r1_ends = np.r_[r1_starts[1:], cfg.NN]
    r1 = np.searchsorted(r1_ends, ni, side="right")

    ord1 = np.lexsort((ei, r1, w1, c1))
    M1, base1, idx1, seg1, scl1 = _group_streams(
        cfg, c1, w1, r1,
        idxval=ni - r1_starts[r1],
        segval=(ei - (c1 * cfg.EPC + w1 * 128)).astype(np.float32),
        sclval=b_inv[ei],
        NRANGE=cfg.R1, NWIN=cfg.EW, order=ord1)

    # per-core per-window scalar columns: binv[128, EW], dinv[128, NW],
    # deg row [1, NW*128] for the degree-scaled bias trick
    bi = np.zeros((C, cfg.EW * 128), np.float32)
    bi[:, :cfg.EPC] = b_inv.reshape(C, cfg.EPC)
    bi = bi.reshape(C, cfg.EW, 128).transpose(0, 2, 1)   # [C,128,EW]
    d_inv1 = np.where(deg_n > 0, 1.0 / deg_n, 1.0).astype(np.float32)
    deg1 = np.maximum(deg_n, 1.0).astype(np.float32)
    di = np.ones((C, cfg.NW * 128), np.float32)
    di[:, :cfg.NPC] = d_inv1.reshape(C, cfg.NPC)
    dg = np.zeros((C, cfg.NW * 128), np.float32)
    dg[:, :cfg.NPC] = deg1.reshape(C, cfg.NPC)           # [C, NW*128]
    di = di.reshape(C, cfg.NW, 128).transpose(0, 2, 1)   # [C,128,NW]

    # ---- stage 2: aggregate ef rows by node (node partition) ----
    # ef rows live in two all-gathered half-tables:
    #   half h of core ce holds its windows [h*EWH0, ...) as rows
    #   ce*EFPh + (w - h_off)*128 + within
    c2 = ni // cfg.NPC
    w2 = (ni % cfg.NPC) // 128
    le = ei % cfg.EPC
    ew = le // 128
    ce = ei // cfg.EPC
    half = (ew >= cfg.EWH0).astype(np.int64)
    efrow = np.where(
        half == 0,
        ce * cfg.EFP0 + ew * 128 + (le % 128),
        ce * cfg.EFP1 + (ew - cfg.EWH0) * 128 + (le % 128))
    r2 = half
    ord2 = np.lexsort((ni, r2, w2, c2))
    M2, base2, idx2, seg2, scl2 = _group_streams(
        cfg, c2, w2, r2,
        idxval=efrow,
        segval=(ni - (c2 * cfg.NPC + w2 * 128)).astype(np.float32),
        sclval=d_inv[ni],
        NRANGE=cfg.R2, NWIN=cfg.NW, order=ord2)

    iota = np.broadcast_to(np.arange(128, dtype=np.float32), (128, 128))
    ident = np.eye(128, dtype=np.float32)
    ones1 = np.ones((1, 128), np.float32)

    in_maps = []
    for c in range(C):
        m = {
            "x_hi": np.ascontiguousarray(x_hi),
            "Wm": np.asarray(W, np.float32).astype(BF16),
            "brow": np.asarray(b, np.float32).astype(BF16).reshape(1, F),
            "ones1": ones1.astype(BF16),
            "iota": iota.astype(BF16).copy(),
            "ident": ident.astype(BF16).copy(),
            "binv": np.ascontiguousarray(bi[c]),
            "dinv": np.ascontiguousarray(di[c]),
            "degrow": np.ascontiguousarray(dg[c].reshape(1, -1).astype(BF16)),
        }
        for r in range(cfg.R1):
            m[f"idx1_{r}"] = _wrap_idx(idx1[r][c][None])[0]
            m[f"seg1_{r}"] = np.ascontiguousarray(
                seg1[r][c].reshape(-1, 128).T.astype(np.float32))
        for r in range(cfg.R2):
            m[f"idx2_{r}"] = _wrap_idx(idx2[r][c][None])[0]
            m[f"seg2_{r}"] = np.ascontiguousarray(
                seg2[r][c].reshape(-1, 128).T.astype(np.float32))
        in_maps.append(m)
    meta = dict(M1=M1, base1=base1, M2=M2, base2=base2,
                r1_starts=[int(v) for v in r1_starts],
                r1_lens=[int(v) for v in (r1_ends - r1_starts)],
                L1=[idx1[r].shape[1] for r in range(cfg.R1)],
                L2=[idx2[r].shape[1] for r in range(cfg.R2)])
    return in_maps, meta


def build_nc(cfg, meta, stages=3):
    import concourse.bacc as bacc
    import concourse.mybir as mybir
    import concourse.tile as tile

    F, C = cfg.F, cfg.C
    M1, base1, M2, base2 = meta["M1"], meta["base1"], meta["M2"], meta["base2"]
    L1, L2 = meta["L1"], meta["L2"]
    r1s, r1l = meta["r1_starts"], meta["r1_lens"]
    f32, bf16, i16 = mybir.dt.float32, mybir.dt.bfloat16, mybir.dt.int16

    nc = bacc.Bacc("TRN2", target_bir_lowering=False, debug=False,
                   num_devices=C, num_swdge_queues=4)

    xhi_d = nc.dram_tensor("x_hi", [cfg.NN, F], bf16, kind="ExternalInput")
    W_d = nc.dram_tensor("Wm", [F, F], bf16, kind="ExternalInput")
    b_d = nc.dram_tensor("brow", [1, F], bf16, kind="ExternalInput")
    ones_d = nc.dram_tensor("ones1", [1, 128], bf16, kind="ExternalInput")
    iota_d = nc.dram_tensor("iota", [128, 128], bf16, kind="ExternalInput")
    ident_d = nc.dram_tensor("ident", [128, 128], bf16, kind="ExternalInput")
    binv_d = nc.dram_tensor("binv", [128, cfg.EW], f32, kind="ExternalInput")
    dinv_d = nc.dram_tensor("dinv", [128, cfg.NW], f32, kind="ExternalInput")
    degr_d = nc.dram_tensor("degrow", [1, cfg.NW * 128], bf16, kind="ExternalInput")
    idx1_d = [nc.dram_tensor(f"idx1_{r}", [128, L1[r] // 16], i16,
                             kind="ExternalInput") for r in range(cfg.R1)]
    seg1_d = [nc.dram_tensor(f"seg1_{r}", [128, L1[r] // 128], f32,
                             kind="ExternalInput") for r in range(cfg.R1)]
    idx2_d = [nc.dram_tensor(f"idx2_{r}", [128, L2[r] // 16], i16,
                             kind="ExternalInput") for r in range(cfg.R2)]
    seg2_d = [nc.dram_tensor(f"seg2_{r}", [128, L2[r] // 128], f32,
                             kind="ExternalInput") for r in range(cfg.R2)]
    out_d = nc.dram_tensor("out", [cfg.NPC, F], f32, kind="ExternalOutput")

    efh_d = [nc.dram_tensor("ef_h0", [cfg.EFP0, F], bf16, kind="Internal"),
             nc.dram_tensor("ef_h1", [cfg.EFP1, F], bf16, kind="Internal")]
    ag_d = [nc.dram_tensor("ef_ag0", [C * cfg.EFP0, F], bf16,
                           kind="Internal", addr_space="Shared"),
            nc.dram_tensor("ef_ag1", [C * cfg.EFP1, F], bf16,
                           kind="Internal", addr_space="Shared")]

    IS_EQ = mybir.AluOpType.is_equal
    COPYF = mybir.ActivationFunctionType.Copy

    qctr = [0]

    def gather_capped(t, src_ap, idx_tile, cbase, span):
        """dma_gather in <=1024-idx calls, round-robin over the 4 SWDGE
        queues (concurrent descriptor generation)."""
        off = 0
        while off < span:
            n = min(1024, span - off)
            nc.gpsimd.dma_gather(
                t[:, off // 128: off // 128 + n // 128, :], src_ap,
                idx_tile[:, cbase * 8 + off // 16: cbase * 8 + (off + n) // 16],
                n, n, F, queue_num=qctr[0] % 4)
            qctr[0] += 1
            off += n

    with tile.TileContext(nc) as tc, ExitStack() as ctx:
        cpool = ctx.enter_context(tc.tile_pool(name="const", bufs=1))
        W_t = cpool.tile([F, F], bf16)
        b_t = cpool.tile([1, F], bf16)
        ones_t = cpool.tile([1, 128], bf16)
        iota_t = cpool.tile([128, 128], bf16)
        ident_t = cpool.tile([128, 128], bf16)
        binv_t = cpool.tile([128, cfg.EW], f32)
        dinv_t = cpool.tile([128, cfg.NW], f32)
        degr_t = cpool.tile([1, cfg.NW * 128], bf16)
        for t, d in ((W_t, W_d), (b_t, b_d), (ones_t, ones_d),
                     (iota_t, iota_d), (ident_t, ident_d),
                     (binv_t, binv_d), (dinv_t, dinv_d), (degr_t, degr_d)):
            nc.sync.dma_start(t[:], d.ap())
        idx1_t, seg1_t = [], []
        for r in range(cfg.R1):
            it = cpool.tile([128, L1[r] // 16], i16, tag=f"i1{r}")
            st = cpool.tile([128, L1[r] // 128], f32, tag=f"s1{r}")
            nc.sync.dma_start(it[:], idx1_d[r].ap())
            nc.sync.dma_start(st[:], seg1_d[r].ap())
            idx1_t.append(it); seg1_t.append(st)
        idx2_t, seg2_t = [], []
        for r in range(cfg.R2):
            it = cpool.tile([128, L2[r] // 16], i16, tag=f"i2{r}")
            st = cpool.tile([128, L2[r] // 128], f32, tag=f"s2{r}")
            nc.sync.dma_start(it[:], idx2_d[r].ap())
            nc.sync.dma_start(st[:], seg2_d[r].ap())
            idx2_t.append(it); seg2_t.append(st)

        efh_v = [efh_d[0].ap().rearrange("(w p) f -> w p f", p=128),
                 efh_d[1].ap().rearrange("(w p) f -> w p f", p=128)]

        # ---------------- stage 1: X rows -> edge features @ W --------------
        with tc.tile_pool(name="g1", bufs=3) as gpool, \
             tc.tile_pool(name="oh1", bufs=16) as ohpool, \
             tc.tile_pool(name="ps1", bufs=4, space="PSUM") as pspool, \
             tc.tile_pool(name="pst", bufs=2, space="PSUM") as ptpool, \
             tc.tile_pool(name="psw", bufs=2, space="PSUM") as pwpool, \
             tc.tile_pool(name="ef1", bufs=4) as efpool:
            for wb in range(0, cfg.EW, cfg.WB1):
                ws = list(range(wb, min(wb + cfg.WB1, cfg.EW)))
                gh, cb = [], []
                for r in range(cfg.R1):
                    nchunks = int(sum(M1[w][r] for w in ws))
                    span = nchunks * 128
                    cbase = int(base1[ws[0]][r]) // 128
                    th = gpool.tile([128, nchunks, F], bf16, tag=f"gh{r}")
                    gather_capped(
                        th, xhi_d.ap()[r1s[r]: r1s[r] + r1l[r], :],
                        idx1_t[r], cbase, span)
                    gh.append(th); cb.append(cbase)
                for w in ws:
                    ps = pspool.tile([128, F], f32, tag="ps")
                    chunks = [(r, m) for r in range(cfg.R1)
                              for m in range(int(M1[w][r]))]
                    for k, (r, m) in enumerate(chunks):
                        gcol = int(base1[w][r]) // 128 + m
                        j = gcol - cb[r]
                        oh = ohpool.tile([128, 128], bf16, tag="oh")
                        nc.vector.tensor_scalar(
                            oh[:], iota_t[:], seg1_t[r][:, gcol:gcol + 1],
                            None, IS_EQ)
                        nc.tensor.matmul(ps[:], oh[:], gh[r][:, j, :],
                                         start=(k == 0), stop=(k == len(chunks) - 1))
                    # ef window (bf16) -> transpose -> @W -> bf16 shard row
                    efb = efpool.tile([128, F], bf16, tag="efb")
                    nc.scalar.activation(efb[:], ps[:], COPYF,
                                         scale=binv_t[:, w:w + 1])
                    pst = ptpool.tile([128, F], bf16, tag="pt")
                    nc.tensor.transpose(pst[:], efb[:], ident_t[:])
                    efT = efpool.tile([128, F], bf16, tag="efT")
                    nc.scalar.copy(efT[:], pst[:])
                    pw = pwpool.tile([128, F], f32, tag="pw")
                    nc.tensor.matmul(pw[:], efT[:], W_t[:], start=True, stop=True)
                    ew_t = efpool.tile([128, F], bf16, tag="ew")
                    nc.scalar.copy(ew_t[:], pw[:])
                    if w < cfg.EWH0:
                        nc.sync.dma_start(efh_v[0][w], ew_t[:])
                    else:
                        nc.sync.dma_start(efh_v[1][w - cfg.EWH0], ew_t[:])
                # dispatch the first-half AllGather as soon as windows
                # [0, EWH0) have been issued, so it overlaps the rest of
                # stage 1
                if stages >= 2 and wb < cfg.EWH0 <= wb + cfg.WB1:
                    nc.gpsimd.collective_compute(
                        "AllGather", mybir.AluOpType.bypass,
                        replica_groups=[list(range(C))],
                        ins=[efh_d[0].ap()], outs=[ag_d[0].ap()])

        # ---------------- exchange second-half edge features -----------------
        if stages >= 2:
            nc.gpsimd.collective_compute(
                "AllGather", mybir.AluOpType.bypass,
                replica_groups=[list(range(C))],
                ins=[efh_d[1].ap()], outs=[ag_d[1].ap()])

        out_v = out_d.ap().rearrange("(w p) f -> w p f", p=128) \
            if cfg.NPC % 128 == 0 else None

        # ---------------- stage 2: edge features -> nodes --------------------
        if stages >= 3:
            batches = [list(range(wb, min(wb + cfg.WB2, cfg.NW)))
                       for wb in range(0, cfg.NW, cfg.WB2)]
            npre = min(cfg.PRE2, len(batches))
            with tc.tile_pool(name="g2p", bufs=1) as ppool, \
                 tc.tile_pool(name="g2", bufs=3) as gpool, \
                 tc.tile_pool(name="oh2", bufs=16) as ohpool, \
                 tc.tile_pool(name="ps2", bufs=4, space="PSUM") as pspool, \
                 tc.tile_pool(name="fin", bufs=4) as fpool:
                pre_tiles = {}
                held_ps = {}

                def issue_gather(bi, r, pool, tag):
                    ws = batches[bi]
                    nchunks = int(sum(M2[w][r] for w in ws))
                    span = nchunks * 128
                    cbase = int(base2[ws[0]][r]) // 128
                    th = pool.tile([128, nchunks, F], bf16, tag=tag)
                    gather_capped(th, ag_d[r].ap(), idx2_t[r], cbase, span)
                    return th, cbase

                def mm_chunks(w, r, ps, cb, gh, first, stop_last=False):
                    nm = int(M2[w][r])
                    for m in range(nm):
                        gcol = int(base2[w][r]) // 128 + m
                        j = gcol - cb
                        oh = ohpool.tile([128, 128], bf16, tag="oh")
                        nc.vector.tensor_scalar(
                            oh[:], iota_t[:], seg2_t[r][:, gcol:gcol + 1],
                            None, IS_EQ)
                        nc.tensor.matmul(ps[:], oh[:], gh[:, j, :],
                                         start=(first and m == 0),
                                         stop=(stop_last and m == nm - 1))

                def window_tail(w, ps):
                    # + deg-scaled bias (so the later dinv scale yields +b)
                    nc.tensor.matmul(
                        ps[:], degr_t[:, w * 128:(w + 1) * 128], b_t[:],
                        start=False, stop=True)
                    ot = fpool.tile([128, F], f32, tag="ot")
                    nc.scalar.activation(ot[:], ps[:], COPYF,
                                         scale=dinv_t[:, w:w + 1])
                    rows = min(128, cfg.NPC - w * 128)
                    if out_v is not None:
                        nc.sync.dma_start(out_v[w], ot[:])
                    else:
                        nc.sync.dma_start(
                            out_d.ap()[w * 128: w * 128 + rows, :],
                            ot[0:rows, :])

                # prologue: all range-0 gathers of the first batches issue
                # before any range-1 gather, and their one-hot matmuls
                # accumulate into held PSUM tiles -- this keeps pool, DVE
                # and PE busy while the second-half AllGather is in flight
                for bi in range(npre):
                    pre_tiles[(bi, 0)] = issue_gather(bi, 0, ppool, f"p{bi}r0")
                for bi in range(npre):
                    pre_tiles[(bi, 1)] = issue_gather(bi, 1, ppool, f"p{bi}r1")
                for bi, ws in enumerate(batches):
                    if bi < npre:
                        gh, cb = zip(*(pre_tiles[(bi, r)] for r in range(cfg.R2)))
                    else:
                        gh, cb = [], []
                        for r in range(cfg.R2):
                            th, cbase = issue_gather(bi, r, gpool, f"gh{r}")
                            gh.append(th); cb.append(cbase)
                    for w in ws:
                        ps = pspool.tile([128, F], f32, tag="ps")
                        for r in range(cfg.R2):
                            mm_chunks(w, r, ps, cb[r], gh[r], first=(r == 0))
                        window_tail(w, ps)

    nc.compile()
    return nc


def _run(cfg, x, hyperedge_index, W, b, trace=False, stages=3, repeats=0):
    import time
    from concourse import bass_utils
    t0 = time.time()
    in_maps, meta = host_prep(cfg, x, hyperedge_index, W, b)
    t1 = time.time()
    nc = build_nc(cfg, meta, stages=stages)
    t2 = time.time()
    res = bass_utils.run_bass_kernel_spmd(
        nc, in_maps, core_ids=list(range(cfg.C)), trace=trace)
    t3 = time.time()
    print(f"[timing] prep={t1-t0:.2f}s build+compile={t2-t1:.2f}s "
          f"first_exec={t3-t2:.2f}s", flush=True)
    for i in range(repeats):
        ta = time.time()
        res = bass_utils.run_bass_kernel_spmd(
            nc, in_maps, core_ids=list(range(cfg.C)), trace=trace)
        print(f"[timing] exec[{i}]={time.time()-ta:.3f}s", flush=True)
    shards = [res.results[c]["out"] for c in range(cfg.C)]
    out = np.concatenate(shards, axis=0).astype(np.float32)
    return out, res


def kernel(x, hyperedge_index, W, b):
    out, _ = _run(FULL, np.asarray(x), np.asarray(hyperedge_index),
                  np.asarray(W), np.asarray(b))
    return out
